# revision 1
# baseline (speedup 1.0000x reference)
"""GQA attention kernel for Trainium2, 8-core tensor-parallel.

Sharding: 8 cores = 2 batches x 4 KV-groups. Each core handles one
(batch, kv_group): projections for its 4 Q-heads + 1 KV-head, RoPE,
causal attention, and its row-shard of Wo -> partial [T, D] output.
Host sums the 4 partials per batch (the Wo all-reduce) at unshard.

Attention runs in transposed orientation: Q^T/K^T live as [HD, T] so
S^T tiles [s,q] come from single K=128 matmuls; softmax row-sums are
computed by an all-ones [128,128] stationary matmul per s-chunk (which
also broadcasts the sums across partitions); 1/sigma (fast approx
reciprocal) scales each head's O^T before the output projection.
Matmuls run in float32r (~2e-4 rel err at ~233ns per 128x128x512 MM).
"""
from contextlib import ExitStack

import numpy as np

import concourse.bass as bass
import concourse.mybir as mybir
import concourse.tile as tile
from concourse import bacc
from concourse.bass_utils import run_bass_kernel_spmd

B, T, D = 2, 2048, 2048
H, KV, HD = 16, 4, 128
R = H // KV                  # 4 query heads per kv head (per core)
GC = R * HD                  # 512 query-proj cols per core
THETA = 10000.0
TQ = 512                     # q-tile size
NJ = T // TQ                 # 4 q-tiles
ND = D // 128                # 16 contraction chunks
SCALE = float(HD) ** -0.5

F32 = mybir.dt.float32
MM_DT = mybir.dt.float32r
MM_NP = np.float32
BF16 = mybir.dt.bfloat16
AF = mybir.ActivationFunctionType

_CACHED_NC = None


def _build_nc():
    nc = bacc.Bacc("TRN2", target_bir_lowering=False, debug=False, num_devices=8)

    xT = nc.dram_tensor("xT", [D, T], MM_DT, kind="ExternalInput").ap()
    wq = nc.dram_tensor("wq", [128, ND * GC], MM_DT, kind="ExternalInput").ap()
    wk = nc.dram_tensor("wk", [128, ND * HD], MM_DT, kind="ExternalInput").ap()
    wv = nc.dram_tensor("wv", [128, ND * HD], MM_DT, kind="ExternalInput").ap()
    wo = nc.dram_tensor("wo", [128, R * D], MM_DT, kind="ExternalInput").ap()
    cosT = nc.dram_tensor("cosT", [HD, T], F32, kind="ExternalInput").ap()
    sinT = nc.dram_tensor("sinT", [HD, T], F32, kind="ExternalInput").ap()
    masks = nc.dram_tensor("masks", [128, 4 * TQ], BF16, kind="ExternalInput").ap()
    out = nc.dram_tensor("out", [T, D], F32, kind="ExternalOutput").ap()

    with tile.TileContext(nc) as tc, ExitStack() as ctx:
        res = ctx.enter_context(tc.tile_pool(name="res", bufs=1))
        sb = ctx.enter_context(tc.tile_pool(name="sb", bufs=2))
        pp = ctx.enter_context(tc.tile_pool(name="pp", bufs=2, space="PSUM"))

        # ---- resident weights / tables ----
        # xt/cos/sin stream on the sync queue; weights go on the scalar /
        # vector engines' queues so j=0's activations aren't stuck behind
        # 10MB of weight DMA.
        # single sync queue, strict priority order: transfers stripe across
        # all 16 DMA engines at full HBM BW, so queue order = arrival order.
        xts0 = []
        for d in range(4):
            xt = sb.tile([128, TQ], MM_DT, tag="xt", bufs=16, name=f"xt0_{d}")
            nc.sync.dma_start(xt[:], xT[d * 128:(d + 1) * 128, 0:TQ])
            xts0.append(xt)
        wk_sb = res.tile([128, ND * HD], MM_DT)
        nc.sync.dma_start(wk_sb[:], wk[:])
        for d in range(4, ND):
            xt = sb.tile([128, TQ], MM_DT, tag="xt", bufs=16, name=f"xt0_{d}")
            nc.sync.dma_start(xt[:], xT[d * 128:(d + 1) * 128, 0:TQ])
            xts0.append(xt)
        cosj0 = sb.tile([128, TQ], F32, tag="cos", bufs=1, name="cosj0")
        nc.sync.dma_start(cosj0[:], cosT[:, 0:TQ])
        sinj0 = sb.tile([128, TQ], F32, tag="sin", bufs=1, name="sinj0")
        nc.sync.dma_start(sinj0[:], sinT[:, 0:TQ])
        wv_sb = res.tile([128, ND * HD], MM_DT)
        nc.sync.dma_start(wv_sb[:], wv[:])
        wq_sb = res.tile([128, ND * GC], MM_DT)    # chunk d at cols [d*GC, (d+1)*GC)
        nc.sync.dma_start(wq_sb[:], wq[:])
        mask_sb = res.tile([128, 4 * TQ], BF16)
        nc.sync.dma_start(mask_sb[:], masks[:])
        wo_sb = res.tile([128, R * D], MM_DT)      # head h rows at cols [h*D, (h+1)*D)
        nc.sync.dma_start(wo_sb[:], wo[:])
        kT_sb = res.tile([128, T], MM_DT)          # K^T resident, filled per j
        v_sb = res.tile([128, T], MM_DT)           # V natural, chunk c at cols c*128
        ident = res.tile([128, 128], F32)
        from concourse.masks import make_identity
        make_identity(nc, ident[:])
        ones_f = res.tile([128, 128], F32)
        nc.vector.memset(ones_f[:], 1.0)
        ones_c = res.tile([128, 128], MM_DT)       # sigma-reduce+broadcast stationary
        nc.vector.tensor_copy(ones_c[:], ones_f[:])

        def rope(dst, ps, cosj, sinj):
            # dst = ps * cos + rotate_half(ps) * sin   (partition dim = head dim)
            rot = sb.tile([128, TQ], F32, tag="rot", bufs=2)
            nc.scalar.mul(rot[0:64, :], ps[64:128, :], -1.0)
            nc.scalar.copy(rot[64:128, :], ps[0:64, :])
            tmp = sb.tile([128, TQ], F32, tag="ropetmp", bufs=2)
            nc.vector.tensor_mul(tmp[:], rot[:], sinj[:])
            nc.vector.tensor_mul(dst, ps[:], cosj[:])
            nc.vector.tensor_add(dst, dst.bitcast(F32), tmp[:])

        for j in range(NJ):
            q0 = j * TQ
            # ---- stage inputs for this q/s tile ----
            if j == 0:
                xts, cosj, sinj = xts0, cosj0, sinj0
            else:
                xts = []
                for d in range(ND):
                    xt = sb.tile([128, TQ], MM_DT, tag="xt", bufs=16)
                    nc.sync.dma_start(xt[:], xT[d * 128:(d + 1) * 128, q0:q0 + TQ])
                    xts.append(xt)
                cosj = sb.tile([128, TQ], F32, tag="cos", bufs=1)
                nc.sync.dma_start(cosj[:], cosT[:, q0:q0 + TQ])
                sinj = sb.tile([128, TQ], F32, tag="sin", bufs=1)
                nc.sync.dma_start(sinj[:], sinT[:, q0:q0 + TQ])

            # ---- A1: K^T and V^T for s-tile j ----
            k_ps = pp.tile([128, TQ], F32, tag="pa", bufs=2)
            vt_ps = pp.tile([128, TQ], F32, tag="pa", bufs=2)
            for d in range(ND):
                nc.tensor.matmul(k_ps[:], wk_sb[:, d * HD:(d + 1) * HD], xts[d][:],
                                 start=(d == 0), stop=(d == ND - 1))
            for d in range(ND):
                nc.tensor.matmul(vt_ps[:], wv_sb[:, d * HD:(d + 1) * HD], xts[d][:],
                                 start=(d == 0), stop=(d == ND - 1))
            rope(kT_sb[:, q0:q0 + TQ], k_ps, cosj, sinj)
            vt_sbt = sb.tile([128, TQ], F32, tag="vtsb", bufs=2)
            nc.vector.tensor_copy(vt_sbt[:], vt_ps[:])
            for c4 in range(4):
                ptt = pp.tile([128, 128], F32, tag="pc", bufs=2)
                nc.tensor.transpose(ptt[:], vt_sbt[:, c4 * 128:(c4 + 1) * 128], ident[:])
                nc.vector.tensor_copy(v_sb[:, (4 * j + c4) * 128:(4 * j + c4 + 1) * 128], ptt[:])

            # ---- A2: Q^T per head + rope ----
            q_tiles = []
            for h in range(R):
                q_ps = pp.tile([128, TQ], F32, tag="pa", bufs=2)
                for d in range(ND):
                    nc.tensor.matmul(
                        q_ps[:], wq_sb[:, d * GC + h * 128:d * GC + (h + 1) * 128],
                        xts[d][:], start=(d == 0), stop=(d == ND - 1))
                qh = sb.tile([128, TQ], MM_DT, tag="qsb", bufs=5)
                rope(qh[:], q_ps, cosj, sinj)
                q_tiles.append(qh)

            # ---- B: causal attention per head ----
            o_tiles = []
            nch = 4 * (j + 1)
            for h in range(R):
                o_ps = pp.tile([128, TQ], F32, tag="po", bufs=2)
                sg_ps = pp.tile([128, TQ], F32, tag="po", bufs=2)
                for c in range(nch):
                    s_ps = pp.tile([128, TQ], F32, tag="ps", bufs=2)
                    nc.tensor.matmul(s_ps[:], kT_sb[:, c * 128:(c + 1) * 128],
                                     q_tiles[h][:], start=True, stop=True)
                    p = sb.tile([128, TQ], MM_DT, tag="psb", bufs=5)
                    nc.scalar.activation(p[:], s_ps[:], AF.Exp, scale=SCALE)
                    if c >= 4 * j:  # diagonal block: apply causal mask
                        m = c - 4 * j
                        nc.vector.tensor_mul(p[:], p[:].bitcast(F32),
                                             mask_sb[:, m * TQ:(m + 1) * TQ])
                    # sigma: ones@p accumulates row-sums broadcast to all parts
                    nc.tensor.matmul(sg_ps[:], ones_c[:], p[:],
                                     start=(c == 0), stop=(c == nch - 1))
                    nc.tensor.matmul(o_ps[:], v_sb[:, c * 128:(c + 1) * 128], p[:],
                                     start=(c == 0), stop=(c == nch - 1))
                sgs = sb.tile([128, TQ], F32, tag="sgs", bufs=2)
                nc.vector.tensor_copy(sgs[:], sg_ps[:])
                rcb = sb.tile([128, TQ], F32, tag="rcb", bufs=2)
                nc.vector.reciprocal_approx_fast(rcb[:], sgs[:])
                oh = sb.tile([128, TQ], MM_DT, tag="osb", bufs=6)
                nc.vector.tensor_mul(oh[:], o_ps[:], rcb[:])
                o_tiles.append(oh)

            # ---- C: output projection for q-tile j ----
            for qs in range(4):
                for n in range(NJ):
                    pc = pp.tile([128, 512], F32, tag="pc", bufs=2)
                    for h in range(R):
                        nc.tensor.matmul(
                            pc[:], o_tiles[h][:, qs * 128:(qs + 1) * 128],
                            wo_sb[:, h * D + n * 512:h * D + (n + 1) * 512],
                            start=(h == 0), stop=(h == R - 1))
                    ob = sb.tile([128, 512], F32, tag="ob", bufs=3)
                    nc.scalar.copy(ob[:], pc[:])
                    nc.gpsimd.dma_start(
                        out[q0 + qs * 128:q0 + (qs + 1) * 128, n * 512:(n + 1) * 512],
                        ob[:])

    nc.compile()
    return nc


def _get_nc():
    global _CACHED_NC
    if _CACHED_NC is None:
        _CACHED_NC = _build_nc()
    return _CACHED_NC


def _rope_tables_T():
    inv_freq = (1.0 / (THETA ** (np.arange(0, HD, 2, dtype=np.float32) / HD))).astype(np.float32)
    pos = np.arange(T, dtype=np.float32)
    freqs = np.outer(pos, inv_freq).astype(np.float32)      # [T, HD/2]
    emb = np.concatenate([freqs, freqs], axis=-1)           # [T, HD]
    return (np.cos(emb).T.astype(np.float32).copy(),
            np.sin(emb).T.astype(np.float32).copy())        # [HD, T]


def _diag_masks():
    # masks[:, m*TQ + jj] for offset delta = m*128: keep jj >= i + delta
    import ml_dtypes
    i = np.arange(128)[:, None]
    jj = np.arange(TQ)[None, :]
    blocks = [(jj >= i + m * 128).astype(ml_dtypes.bfloat16) for m in range(4)]
    return np.concatenate(blocks, axis=1)                   # [128, 4*TQ]


def kernel(x, Wq, Wk, Wv, Wo, _trace=False):
    x = np.asarray(x, dtype=np.float32)
    Wq = np.asarray(Wq, dtype=MM_NP)
    Wk = np.asarray(Wk, dtype=MM_NP)
    Wv = np.asarray(Wv, dtype=MM_NP)
    Wo = np.asarray(Wo, dtype=MM_NP)

    cosT, sinT = _rope_tables_T()
    masks = _diag_masks()
    in_maps = []
    for core in range(8):
        b, g = core // KV, core % KV
        def chunkT(w):  # [ND*128, C] -> [128, ND*C] with chunk d at cols [d*C,(d+1)*C)
            nd = w.shape[0] // 128
            return np.ascontiguousarray(
                w.reshape(nd, 128, -1).transpose(1, 0, 2).reshape(128, -1))
        in_maps.append({
            "xT": np.ascontiguousarray(x[b].T.astype(MM_NP)),
            "wq": chunkT(Wq[:, g * GC:(g + 1) * GC]),
            "wk": chunkT(Wk[:, g * HD:(g + 1) * HD]),
            "wv": chunkT(Wv[:, g * HD:(g + 1) * HD]),
            "wo": chunkT(Wo[g * GC:(g + 1) * GC, :]),
            "cosT": cosT, "sinT": sinT, "masks": masks,
        })

    nc = _get_nc()
    res = run_bass_kernel_spmd(nc, in_maps, core_ids=list(range(8)), trace=_trace)

    outp = np.zeros((B, T, D), dtype=np.float32)
    for core in range(8):
        b = core // KV
        outp[b] += res.results[core]["out"]
    if _trace:
        kernel._last_exec_time_ns = res.exec_time_ns
        kernel._last_trace = res.instructions_and_trace
    return outp



# revision 9
# speedup vs baseline: 1.1583x; 1.1583x over previous
"""GQA attention kernel for Trainium2, 8-core tensor-parallel.

Sharding: 8 cores = 2 batches x 4 KV-groups. Each core handles one
(batch, kv_group): projections for its 4 Q-heads + 1 KV-head, RoPE,
causal attention, and its row-shard of Wo -> partial [T, D] output.
Host sums the 4 partials per batch (the Wo all-reduce) at unshard.

V2: fp16 matmul operands throughout (same PE rate as f32r, half the
DMA/SBUF, 4x DVE modes). Attention in transposed orientation (S^T
tiles [s,q] from single K=128 matmuls). Softmax row-sums accumulate on
the vector engine into an SBUF f32 tile, reduced by ONE all-ones
stationary matmul per (j,head) which also broadcasts sums across
partitions. Diagonal blocks are trimmed to the causal region at
128-col granularity; the per-element causal mask is a single [128,128]
triangle multiply per diagonal block. exp() runs once per PAIR of
s-chunks over a 2-bank PSUM tile. PSUM->SBUF output copies run on the
Pool engine; output partials are written fp16 (host sums in f32).
"""
from contextlib import ExitStack

import numpy as np

import concourse.bass as bass
import concourse.mybir as mybir
import concourse.tile as tile
from concourse import bacc
from concourse.bass_utils import run_bass_kernel_spmd

B, T, D = 2, 2048, 2048
H, KV, HD = 16, 4, 128
R = H // KV                  # 4 query heads per kv head (per core)
GC = R * HD                  # 512 query-proj cols per core
THETA = 10000.0
TQ = 512                     # q-tile size
NJ = T // TQ                 # 4 q-tiles
ND = D // 128                # 16 contraction chunks
SCALE = float(HD) ** -0.5

F32 = mybir.dt.float32
F32R = mybir.dt.float32r
FP16 = mybir.dt.float16
AF = mybir.ActivationFunctionType

_CACHED_NC = None


def _build_nc():
    nc = bacc.Bacc("TRN2", target_bir_lowering=False, debug=False, num_devices=8)

    xT = nc.dram_tensor("xT", [ND, 128, T], FP16, kind="ExternalInput").ap()
    wq = nc.dram_tensor("wq", [128, ND * GC], FP16, kind="ExternalInput").ap()
    wk = nc.dram_tensor("wk", [128, ND * HD], FP16, kind="ExternalInput").ap()
    wv = nc.dram_tensor("wv", [128, ND * HD], FP16, kind="ExternalInput").ap()
    wo = nc.dram_tensor("wo", [128, R * D], FP16, kind="ExternalInput").ap()
    cosT = nc.dram_tensor("cosT", [HD, T], FP16, kind="ExternalInput").ap()
    sinT = nc.dram_tensor("sinT", [HD, T], FP16, kind="ExternalInput").ap()
    trimask = nc.dram_tensor("trimask", [128, 128], FP16, kind="ExternalInput").ap()
    out = nc.dram_tensor("out", [T, D], FP16, kind="ExternalOutput").ap()

    with tile.TileContext(nc) as tc, ExitStack() as ctx:
        res = ctx.enter_context(tc.tile_pool(name="res", bufs=1))
        sb = ctx.enter_context(tc.tile_pool(name="sb", bufs=2))
        pp = ctx.enter_context(tc.tile_pool(name="pp", bufs=2, space="PSUM"))

        # ---- resident weights / tables ----
        # j=0 activations interleave with the weight DMAs in consumption
        # order so the first matmuls start as early as possible.
        xt0_q = []
        for qtr in range(4):
            xq = sb.tile([128, 4, TQ], FP16, tag="xtq", bufs=4, name=f"xt0_q{qtr}")
            # 3D gather: chunk d lives at xT[d, :, :], cols [0,TQ)
            nc.sync.dma_start(
                xq[:], xT[qtr * 4:(qtr + 1) * 4, :, 0:TQ].transpose([1, 0, 2]))
            xt0_q.append(xq)
        wk_sb = res.tile([128, ND * HD], FP16)
        nc.sync.dma_start(wk_sb[:], wk[:])
        cosj0 = sb.tile([128, TQ], FP16, tag="cos", bufs=2, name="cosj0")
        nc.sync.dma_start(cosj0[:], cosT[:, 0:TQ])
        sinj0 = sb.tile([128, TQ], FP16, tag="sin", bufs=2, name="sinj0")
        nc.sync.dma_start(sinj0[:], sinT[:, 0:TQ])
        wv_sb = res.tile([128, ND * HD], FP16)
        nc.sync.dma_start(wv_sb[:], wv[:])
        wq_sb = res.tile([128, ND * GC], FP16)    # chunk d at cols [d*GC, (d+1)*GC)
        nc.sync.dma_start(wq_sb[:], wq[:])
        mask_sb = res.tile([128, 128], FP16)
        nc.sync.dma_start(mask_sb[:], trimask[:])
        wo_sb = res.tile([128, R * D], FP16)      # head h rows at cols [h*D, (h+1)*D)
        nc.sync.dma_start(wo_sb[:], wo[:])
        kT_sb = res.tile([128, T], FP16)          # K^T resident, filled per j
        v_sb = res.tile([128, T], FP16)           # V natural, chunk c at cols c*128
        ident = res.tile([128, 128], FP16)
        from concourse.masks import make_identity
        make_identity(nc, ident[:])
        ones_f = res.tile([128, 128], F32)
        nc.vector.memset(ones_f[:], 1.0)
        ones_c = res.tile([128, 128], F32R)       # sigma-reduce+broadcast stationary
        nc.vector.tensor_copy(ones_c[:], ones_f[:])

        def rope(dst, ps, cosj, sinj):
            # dst = ps * cos + rotate_half(ps) * sin   (partition dim = head dim)
            # one scalar op moves PSUM->SBUF fp16; the rest is 4x-mode DVE.
            ps_sb = sb.tile([128, TQ], FP16, tag="ps_sb", bufs=2)
            nc.scalar.copy(ps_sb[:], ps[:])
            rot = sb.tile([128, TQ], FP16, tag="rot", bufs=2)
            nc.vector.tensor_scalar_mul(rot[0:64, :], ps_sb[64:128, :], -1.0)
            nc.vector.tensor_copy(rot[64:128, :], ps_sb[0:64, :])
            tmp = sb.tile([128, TQ], FP16, tag="ropetmp", bufs=2)
            nc.vector.tensor_mul(tmp[:], rot[:], sinj[:])
            nc.vector.tensor_mul(dst, ps_sb[:], cosj[:])
            nc.vector.tensor_add(dst, dst, tmp[:])

        for j in range(NJ):
            q0 = j * TQ
            # ---- stage inputs for this q/s tile ----
            if j == 0:
                xts = [xt0_q[d // 4][:, d % 4, :] for d in range(ND)]
                cosj, sinj = cosj0, sinj0
            else:
                xt_all = sb.tile([128, ND, TQ], FP16, tag="xt", bufs=2)
                nc.sync.dma_start(
                    xt_all[:], xT[:, :, q0:q0 + TQ].transpose([1, 0, 2]))
                xts = [xt_all[:, d, :] for d in range(ND)]
                cosj = sb.tile([128, TQ], FP16, tag="cos", bufs=2)
                nc.sync.dma_start(cosj[:], cosT[:, q0:q0 + TQ])
                sinj = sb.tile([128, TQ], FP16, tag="sin", bufs=2)
                nc.sync.dma_start(sinj[:], sinT[:, q0:q0 + TQ])

            # ---- A1: K^T and V^T for s-tile j ----
            k_ps = pp.tile([128, TQ], F32, tag="pa", bufs=2)
            vt_ps = pp.tile([128, TQ], F32, tag="pa", bufs=2)
            for d in range(ND):
                nc.tensor.matmul(k_ps[:], wk_sb[:, d * HD:(d + 1) * HD], xts[d],
                                 start=(d == 0), stop=(d == ND - 1))
            for d in range(ND):
                nc.tensor.matmul(vt_ps[:], wv_sb[:, d * HD:(d + 1) * HD], xts[d],
                                 start=(d == 0), stop=(d == ND - 1))
            rope(kT_sb[:, q0:q0 + TQ], k_ps, cosj, sinj)
            vt_sbt = sb.tile([128, TQ], FP16, tag="vtsb", bufs=2)
            nc.scalar.copy(vt_sbt[:], vt_ps[:])
            for c4 in range(4):
                ptt = pp.tile([128, 128], FP16, tag="po", bufs=2)
                nc.tensor.transpose(ptt[:], vt_sbt[:, c4 * 128:(c4 + 1) * 128], ident[:])
                nc.vector.tensor_copy(v_sb[:, (4 * j + c4) * 128:(4 * j + c4 + 1) * 128], ptt[:])

            # ---- A2: Q^T per head + rope ----
            q_tiles = []
            for h in range(R):
                q_ps = pp.tile([128, TQ], F32, tag="pa", bufs=2)
                for d in range(ND):
                    nc.tensor.matmul(
                        q_ps[:], wq_sb[:, d * GC + h * 128:d * GC + (h + 1) * 128],
                        xts[d], start=(d == 0), stop=(d == ND - 1))
                qh = sb.tile([128, TQ], FP16, tag="qsb", bufs=5)
                rope(qh[:], q_ps, cosj, sinj)
                q_tiles.append(qh)

            # ---- B: causal attention per head (transposed S^T orientation) ----
            o_tiles = []
            nch = 4 * (j + 1)
            npair = nch // 2
            for h in range(R):
                o_ps = pp.tile([128, TQ], F32, tag="po", bufs=2)
                acc = sb.tile([128, TQ], F32R, tag="acc", bufs=2)
                for i in range(npair):
                    pair = ((0, 2 * i), (1, 2 * i + 1))
                    sp = pp.tile([128, 2 * TQ], F32, tag="sp", bufs=2)
                    p_sb = sb.tile([128, 2 * TQ], FP16, tag="psb", bufs=4)
                    for k, c in pair:
                        m = c - 4 * j
                        lo = m * 128 if m > 0 else 0
                        nc.tensor.matmul(sp[:, k * TQ + lo:(k + 1) * TQ],
                                         kT_sb[:, c * 128:(c + 1) * 128],
                                         q_tiles[h][:, lo:TQ], start=True, stop=True)
                    if pair[1][1] < 4 * j:  # both off-diagonal: one wide exp
                        nc.scalar.activation(p_sb[:], sp[:], AF.Exp, scale=SCALE)
                    else:
                        for k, c in pair:
                            m = c - 4 * j
                            lo = m * 128 if m > 0 else 0
                            nc.scalar.activation(p_sb[:, k * TQ + lo:(k + 1) * TQ],
                                                 sp[:, k * TQ + lo:(k + 1) * TQ],
                                                 AF.Exp, scale=SCALE)
                    for k, c in pair:
                        m = c - 4 * j
                        if m >= 0:  # diagonal block: triangle mask
                            blk = slice(k * TQ + m * 128, k * TQ + (m + 1) * 128)
                            nc.vector.tensor_mul(p_sb[:, blk], p_sb[:, blk], mask_sb[:])
                    # sigma: accumulate exp rows on DVE (partition-reduced later)
                    for k, c in pair:
                        m = c - 4 * j
                        lo = m * 128 if m > 0 else 0
                        if i == 0 and k == 0:
                            nc.vector.tensor_copy(acc[:], p_sb[:, 0:TQ])
                        else:
                            nc.vector.tensor_add(acc[:, lo:TQ], acc[:, lo:TQ],
                                                 p_sb[:, k * TQ + lo:(k + 1) * TQ])
                    for k, c in pair:
                        m = c - 4 * j
                        lo = m * 128 if m > 0 else 0
                        nc.tensor.matmul(o_ps[:, lo:TQ], v_sb[:, c * 128:(c + 1) * 128],
                                         p_sb[:, k * TQ + lo:(k + 1) * TQ],
                                         start=(i == 0 and k == 0),
                                         stop=(i == npair - 1 and k == 1))
                # partition-reduce sigma + broadcast via all-ones stationary
                sg_ps = pp.tile([128, TQ], F32, tag="po", bufs=2)
                nc.tensor.matmul(sg_ps[:], ones_c[:], acc[:], start=True, stop=True)
                rcb = sb.tile([128, TQ], F32, tag="rcb", bufs=2)
                nc.vector.reciprocal_approx_fast(rcb[:], sg_ps[:])
                oh = sb.tile([128, TQ], FP16, tag="osb", bufs=6)
                nc.vector.tensor_mul(oh[:], o_ps[:], rcb[:])
                o_tiles.append(oh)

            # ---- C: output projection for q-tile j ----
            for qs in range(4):
                ob = sb.tile([128, D], FP16, tag="ob", bufs=3)
                for n in range(NJ):
                    pc = pp.tile([128, 512], F32, tag="pa", bufs=2)
                    for h in range(R):
                        nc.tensor.matmul(
                            pc[:], o_tiles[h][:, qs * 128:(qs + 1) * 128],
                            wo_sb[:, h * D + n * 512:h * D + (n + 1) * 512],
                            start=(h == 0), stop=(h == R - 1))
                    if n % 2 == 0:
                        nc.scalar.copy(ob[:, n * 512:(n + 1) * 512], pc[:])
                    else:
                        nc.vector.tensor_copy(ob[:, n * 512:(n + 1) * 512], pc[:])
                nc.gpsimd.dma_start(
                    out[q0 + qs * 128:q0 + (qs + 1) * 128, :], ob[:])

    nc.compile()
    return nc


def _get_nc():
    global _CACHED_NC
    if _CACHED_NC is None:
        _CACHED_NC = _build_nc()
    return _CACHED_NC


def _rope_tables_T():
    inv_freq = (1.0 / (THETA ** (np.arange(0, HD, 2, dtype=np.float32) / HD))).astype(np.float32)
    pos = np.arange(T, dtype=np.float32)
    freqs = np.outer(pos, inv_freq).astype(np.float32)      # [T, HD/2]
    emb = np.concatenate([freqs, freqs], axis=-1)           # [T, HD]
    return (np.cos(emb).T.astype(np.float16).copy(),
            np.sin(emb).T.astype(np.float16).copy())        # [HD, T]


def _tri_mask():
    # keep col >= row within a 128x128 diagonal block
    i = np.arange(128)[:, None]
    jj = np.arange(128)[None, :]
    return (jj >= i).astype(np.float16)


def kernel(x, Wq, Wk, Wv, Wo, _trace=False):
    x = np.asarray(x, dtype=np.float32)
    Wq = np.asarray(Wq, dtype=np.float16)
    Wk = np.asarray(Wk, dtype=np.float16)
    Wv = np.asarray(Wv, dtype=np.float16)
    Wo = np.asarray(Wo, dtype=np.float16)

    cosT, sinT = _rope_tables_T()
    trimask = _tri_mask()
    in_maps = []
    for core in range(8):
        b, g = core // KV, core % KV
        def chunkT(w):  # [ND*128, C] -> [128, ND*C] with chunk d at cols [d*C,(d+1)*C)
            nd = w.shape[0] // 128
            return np.ascontiguousarray(
                w.reshape(nd, 128, -1).transpose(1, 0, 2).reshape(128, -1))
        in_maps.append({
            "xT": np.ascontiguousarray(x[b].T.astype(np.float16)).reshape(ND, 128, T),
            "wq": chunkT(Wq[:, g * GC:(g + 1) * GC]),
            "wk": chunkT(Wk[:, g * HD:(g + 1) * HD]),
            "wv": chunkT(Wv[:, g * HD:(g + 1) * HD]),
            "wo": chunkT(Wo[g * GC:(g + 1) * GC, :]),
            "cosT": cosT, "sinT": sinT, "trimask": trimask,
        })

    nc = _get_nc()
    res = run_bass_kernel_spmd(nc, in_maps, core_ids=list(range(8)), trace=_trace)

    outp = np.zeros((B, T, D), dtype=np.float32)
    for core in range(8):
        b = core // KV
        outp[b] += res.results[core]["out"].astype(np.float32)
    if _trace:
        kernel._last_exec_time_ns = res.exec_time_ns
        kernel._last_trace = res.instructions_and_trace
    return outp


# revision 15
# speedup vs baseline: 1.5204x; 1.3127x over previous
"""GQA attention kernel for Trainium2, 8-core tensor-parallel.

Sharding: 8 cores = 2 batches x 4 KV-groups. Each core handles one
(batch, kv_group): projections for its 4 Q-heads + 1 KV-head, RoPE,
causal attention, and its row-shard of Wo -> partial [T, D] output.
Host sums the 4 partials per batch (the Wo all-reduce) at unshard.

V2: fp16 matmul operands throughout (same PE rate as f32r, half the
DMA/SBUF, 4x DVE modes). Attention in transposed orientation (S^T
tiles [s,q] from single K=128 matmuls). Softmax row-sums accumulate on
the vector engine into an SBUF f32 tile, reduced by ONE all-ones
stationary matmul per (j,head) which also broadcasts sums across
partitions. Diagonal blocks are trimmed to the causal region at
128-col granularity; the per-element causal mask is a single [128,128]
triangle multiply per diagonal block. exp() runs once per PAIR of
s-chunks over a 2-bank PSUM tile. PSUM->SBUF output copies run on the
Pool engine; output partials are written fp16 (host sums in f32).
"""
from contextlib import ExitStack

import numpy as np

import concourse.bass as bass
import concourse.mybir as mybir
import concourse.tile as tile
from concourse import bacc
from concourse.bass_utils import run_bass_kernel_spmd

B, T, D = 2, 2048, 2048
H, KV, HD = 16, 4, 128
R = H // KV                  # 4 query heads per kv head (per core)
GC = R * HD                  # 512 query-proj cols per core
THETA = 10000.0
TQ = 512                     # q-tile size
NJ = T // TQ                 # 4 q-tiles
ND = D // 128                # 16 contraction chunks
SCALE = float(HD) ** -0.5

F32 = mybir.dt.float32
F32R = mybir.dt.float32r
FP16 = mybir.dt.float16
AF = mybir.ActivationFunctionType

_CACHED_NC = None


def _build_nc():
    nc = bacc.Bacc("TRN2", target_bir_lowering=False, debug=False, num_devices=8)

    xT = nc.dram_tensor("xT", [ND, 128, T], FP16, kind="ExternalInput").ap()
    wq = nc.dram_tensor("wq", [128, ND * GC], FP16, kind="ExternalInput").ap()
    wk = nc.dram_tensor("wk", [128, ND * HD], FP16, kind="ExternalInput").ap()
    wv = nc.dram_tensor("wv", [128, ND * HD], FP16, kind="ExternalInput").ap()
    wo = nc.dram_tensor("wo", [128, R * D], FP16, kind="ExternalInput").ap()
    cosT = nc.dram_tensor("cosT", [HD, T], FP16, kind="ExternalInput").ap()
    sinT = nc.dram_tensor("sinT", [HD, T], FP16, kind="ExternalInput").ap()
    trimask = nc.dram_tensor("trimask", [128, 128], FP16, kind="ExternalInput").ap()
    out = nc.dram_tensor("out", [T, D], FP16, kind="ExternalOutput").ap()

    with tile.TileContext(nc) as tc, ExitStack() as ctx:
        res = ctx.enter_context(tc.tile_pool(name="res", bufs=1))
        sb = ctx.enter_context(tc.tile_pool(name="sb", bufs=2))
        pp = ctx.enter_context(tc.tile_pool(name="pp", bufs=2, space="PSUM"))

        # ---- resident weights / tables ----
        # j=0 activations interleave with the weight DMAs in consumption
        # order so the first matmuls start as early as possible.
        xt0_q = [sb.tile([128, 4, TQ], FP16, tag="xtq", bufs=4, name=f"xt0_q{qtr}")
                 for qtr in range(4)]

        def load_xq(qtr):  # 3D gather: chunk d lives at xT[d, :, :], cols [0,TQ)
            nc.sync.dma_start(
                xt0_q[qtr][:], xT[qtr * 4:(qtr + 1) * 4, :, 0:TQ].transpose([1, 0, 2]))

        load_xq(0)
        wk_sb = res.tile([128, ND * HD], FP16)
        nc.sync.dma_start(wk_sb[:], wk[:])
        load_xq(1)
        load_xq(2)
        load_xq(3)
        cosj0 = sb.tile([128, TQ], FP16, tag="cos", bufs=2, name="cosj0")
        nc.sync.dma_start(cosj0[:], cosT[:, 0:TQ])
        sinj0 = sb.tile([128, TQ], FP16, tag="sin", bufs=2, name="sinj0")
        nc.sync.dma_start(sinj0[:], sinT[:, 0:TQ])
        wv_sb = res.tile([128, ND * HD], FP16)
        nc.sync.dma_start(wv_sb[:], wv[:])
        wq_sb = res.tile([128, ND * GC], FP16)    # chunk d at cols [d*GC, (d+1)*GC)
        nc.sync.dma_start(wq_sb[:], wq[:])
        mask_sb = res.tile([128, 128], FP16)
        nc.sync.dma_start(mask_sb[:], trimask[:])
        wo_sb = res.tile([128, R * D], FP16)      # head h rows at cols [h*D, (h+1)*D)
        nc.sync.dma_start(wo_sb[:], wo[:])
        kT_sb = res.tile([128, T], FP16)          # K^T resident, filled per j
        v_sb = res.tile([128, T], FP16)           # V natural, chunk c at cols c*128
        ident = res.tile([128, 128], FP16)
        from concourse.masks import make_identity
        make_identity(nc, ident[:])
        ones_c = res.tile([128, 128], FP16)       # sigma-reduce+broadcast stationary
        nc.vector.memset(ones_c[:], 1.0)

        def c_block(o_t, q0p, qs):
            # output projection for rows [q0p+qs*128, q0p+(qs+1)*128)
            ob = sb.tile([128, D], FP16, tag="ob", bufs=3)
            for n in range(NJ):
                pc = pp.tile([128, 512], F32, tag="pa", bufs=2)
                for h2 in range(R):
                    nc.tensor.matmul(
                        pc[:], o_t[h2][:, qs * 128:(qs + 1) * 128],
                        wo_sb[:, h2 * D + n * 512:h2 * D + (n + 1) * 512],
                        start=(h2 == 0), stop=(h2 == R - 1))
                if n % 2 == 0:
                    nc.scalar.copy(ob[:, n * 512:(n + 1) * 512], pc[:])
                else:
                    nc.vector.tensor_copy(ob[:, n * 512:(n + 1) * 512], pc[:])
            nc.gpsimd.dma_start(
                out[q0p + qs * 128:q0p + (qs + 1) * 128, :], ob[:])

        def rope(dst, ps, cosj, sinj):
            # dst = ps * cos + rotate_half(ps) * sin   (partition dim = head dim)
            # one scalar op moves PSUM->SBUF fp16; the rest is 4x-mode DVE.
            ps_sb = sb.tile([128, TQ], FP16, tag="ps_sb", bufs=2)
            nc.scalar.copy(ps_sb[:], ps[:])
            rot = sb.tile([128, TQ], FP16, tag="rot", bufs=2)
            nc.vector.tensor_scalar_mul(rot[0:64, :], ps_sb[64:128, :], -1.0)
            nc.vector.tensor_copy(rot[64:128, :], ps_sb[0:64, :])
            tmp = sb.tile([128, TQ], FP16, tag="ropetmp", bufs=2)
            nc.vector.tensor_mul(tmp[:], rot[:], sinj[:])
            nc.vector.tensor_mul(dst, ps_sb[:], cosj[:])
            nc.vector.tensor_add(dst, dst, tmp[:])

        prev_o, prev_q0 = None, 0
        for j in range(NJ):
            q0 = j * TQ
            # ---- stage inputs for this q/s tile ----
            if j == 0:
                xts = [xt0_q[d // 4][:, d % 4, :] for d in range(ND)]
                cosj, sinj = cosj0, sinj0
            else:
                xt_all = sb.tile([128, ND, TQ], FP16, tag="xt", bufs=2)
                nc.sync.dma_start(
                    xt_all[:], xT[:, :, q0:q0 + TQ].transpose([1, 0, 2]))
                xts = [xt_all[:, d, :] for d in range(ND)]
                cosj = sb.tile([128, TQ], FP16, tag="cos", bufs=2)
                nc.sync.dma_start(cosj[:], cosT[:, q0:q0 + TQ])
                sinj = sb.tile([128, TQ], FP16, tag="sin", bufs=2)
                nc.sync.dma_start(sinj[:], sinT[:, q0:q0 + TQ])

            # ---- A1: K^T and V^T for s-tile j ----
            k_ps = pp.tile([128, TQ], F32, tag="pa", bufs=2)
            vt_ps = pp.tile([128, TQ], F32, tag="pa", bufs=2)
            for d in range(ND):
                nc.tensor.matmul(k_ps[:], wk_sb[:, d * HD:(d + 1) * HD], xts[d],
                                 start=(d == 0), stop=(d == ND - 1))
            for d in range(ND):
                nc.tensor.matmul(vt_ps[:], wv_sb[:, d * HD:(d + 1) * HD], xts[d],
                                 start=(d == 0), stop=(d == ND - 1))
            rope(kT_sb[:, q0:q0 + TQ], k_ps, cosj, sinj)
            vt_sbt = sb.tile([128, TQ], FP16, tag="vtsb", bufs=2)
            nc.scalar.copy(vt_sbt[:], vt_ps[:])
            for c4 in range(4):
                ptt = pp.tile([128, 128], FP16, tag="po", bufs=2)
                nc.tensor.transpose(ptt[:], vt_sbt[:, c4 * 128:(c4 + 1) * 128], ident[:])
                nc.vector.tensor_copy(v_sb[:, (4 * j + c4) * 128:(4 * j + c4 + 1) * 128], ptt[:])

            # ---- A2: Q^T per head + rope ----
            q_tiles = []
            for h in range(R):
                q_ps = pp.tile([128, TQ], F32, tag="pa", bufs=2)
                for d in range(ND):
                    nc.tensor.matmul(
                        q_ps[:], wq_sb[:, d * GC + h * 128:d * GC + (h + 1) * 128],
                        xts[d], start=(d == 0), stop=(d == ND - 1))
                qh = sb.tile([128, TQ], FP16, tag="qsb", bufs=5)
                rope(qh[:], q_ps, cosj, sinj)
                q_tiles.append(qh)

            # ---- B: causal attention per head (transposed S^T orientation),
            # interleaved with the previous tile's output projection so the
            # tensor engine has scalar-independent work during exp waits ----
            o_tiles = []
            nch = 4 * (j + 1)
            npair = nch // 2
            for h in range(R):
                o_ps = pp.tile([128, TQ], F32, tag="po", bufs=2)
                acc = sb.tile([128, TQ], FP16, tag="acc", bufs=2)
                for i in range(npair):
                    pair = ((0, 2 * i), (1, 2 * i + 1))
                    sp = pp.tile([128, 2 * TQ], F32, tag="sp", bufs=2)
                    p_sb = sb.tile([128, 2 * TQ], FP16, tag="psb", bufs=4)
                    for k, c in pair:
                        m = c - 4 * j
                        lo = m * 128 if m > 0 else 0
                        nc.tensor.matmul(sp[:, k * TQ + lo:(k + 1) * TQ],
                                         kT_sb[:, c * 128:(c + 1) * 128],
                                         q_tiles[h][:, lo:TQ], start=True, stop=True)
                    if pair[1][1] < 4 * j:  # both off-diagonal: one wide exp
                        nc.scalar.activation(p_sb[:], sp[:], AF.Exp, scale=SCALE)
                    else:
                        for k, c in pair:
                            m = c - 4 * j
                            lo = m * 128 if m > 0 else 0
                            nc.scalar.activation(p_sb[:, k * TQ + lo:(k + 1) * TQ],
                                                 sp[:, k * TQ + lo:(k + 1) * TQ],
                                                 AF.Exp, scale=SCALE)
                    for k, c in pair:
                        m = c - 4 * j
                        if m >= 0:  # diagonal block: triangle mask
                            blk = slice(k * TQ + m * 128, k * TQ + (m + 1) * 128)
                            nc.vector.tensor_mul(p_sb[:, blk], p_sb[:, blk], mask_sb[:])
                    # sigma: accumulate exp rows on DVE (partition-reduced later)
                    for k, c in pair:
                        m = c - 4 * j
                        lo = m * 128 if m > 0 else 0
                        if i == 0 and k == 0:
                            nc.vector.tensor_copy(acc[:], p_sb[:, 0:TQ])
                        else:
                            nc.vector.tensor_add(acc[:, lo:TQ], acc[:, lo:TQ],
                                                 p_sb[:, k * TQ + lo:(k + 1) * TQ])
                    for k, c in pair:
                        m = c - 4 * j
                        lo = m * 128 if m > 0 else 0
                        nc.tensor.matmul(o_ps[:, lo:TQ], v_sb[:, c * 128:(c + 1) * 128],
                                         p_sb[:, k * TQ + lo:(k + 1) * TQ],
                                         start=(i == 0 and k == 0),
                                         stop=(i == npair - 1 and k == 1))
                # partition-reduce sigma + broadcast via all-ones stationary
                sg_ps = pp.tile([128, TQ], F32, tag="po", bufs=2)
                nc.tensor.matmul(sg_ps[:], ones_c[:], acc[:], start=True, stop=True)
                rcb = sb.tile([128, TQ], F32, tag="rcb", bufs=2)
                nc.vector.reciprocal_approx_fast(rcb[:], sg_ps[:])
                oh = sb.tile([128, TQ], FP16, tag="osb", bufs=8)
                nc.vector.tensor_mul(oh[:], o_ps[:], rcb[:])
                o_tiles.append(oh)
                # previous q-tile's output projection: pure-tensor filler
                if prev_o is not None:
                    c_block(prev_o, prev_q0, h)
            prev_o, prev_q0 = o_tiles, q0

        # ---- C for the last q-tile ----
        for qs in range(4):
            c_block(prev_o, prev_q0, qs)

    nc.compile()
    return nc


def _get_nc():
    global _CACHED_NC
    if _CACHED_NC is None:
        _CACHED_NC = _build_nc()
    return _CACHED_NC


def _rope_tables_T():
    inv_freq = (1.0 / (THETA ** (np.arange(0, HD, 2, dtype=np.float32) / HD))).astype(np.float32)
    pos = np.arange(T, dtype=np.float32)
    freqs = np.outer(pos, inv_freq).astype(np.float32)      # [T, HD/2]
    emb = np.concatenate([freqs, freqs], axis=-1)           # [T, HD]
    return (np.cos(emb).T.astype(np.float16).copy(),
            np.sin(emb).T.astype(np.float16).copy())        # [HD, T]


def _tri_mask():
    # keep col >= row within a 128x128 diagonal block
    i = np.arange(128)[:, None]
    jj = np.arange(128)[None, :]
    return (jj >= i).astype(np.float16)


def kernel(x, Wq, Wk, Wv, Wo, _trace=False):
    x = np.asarray(x, dtype=np.float32)
    Wq = np.asarray(Wq, dtype=np.float16)
    Wk = np.asarray(Wk, dtype=np.float16)
    Wv = np.asarray(Wv, dtype=np.float16)
    Wo = np.asarray(Wo, dtype=np.float16)

    cosT, sinT = _rope_tables_T()
    trimask = _tri_mask()
    in_maps = []
    for core in range(8):
        b, g = core // KV, core % KV
        def chunkT(w):  # [ND*128, C] -> [128, ND*C] with chunk d at cols [d*C,(d+1)*C)
            nd = w.shape[0] // 128
            return np.ascontiguousarray(
                w.reshape(nd, 128, -1).transpose(1, 0, 2).reshape(128, -1))
        in_maps.append({
            "xT": np.ascontiguousarray(x[b].T.astype(np.float16)).reshape(ND, 128, T),
            "wq": chunkT(Wq[:, g * GC:(g + 1) * GC]),
            "wk": chunkT(Wk[:, g * HD:(g + 1) * HD]),
            "wv": chunkT(Wv[:, g * HD:(g + 1) * HD]),
            "wo": chunkT(Wo[g * GC:(g + 1) * GC, :]),
            "cosT": cosT, "sinT": sinT, "trimask": trimask,
        })

    nc = _get_nc()
    res = run_bass_kernel_spmd(nc, in_maps, core_ids=list(range(8)), trace=_trace)

    outp = np.zeros((B, T, D), dtype=np.float32)
    for core in range(8):
        b = core // KV
        outp[b] += res.results[core]["out"].astype(np.float32)
    if _trace:
        kernel._last_exec_time_ns = res.exec_time_ns
        kernel._last_trace = res.instructions_and_trace
    return outp
